# revision 1
# baseline (speedup 1.0000x reference)
"""GATv2 layer on 8 Trainium2 NeuronCores (Bass/Tile).

Self-contained: takes full inputs, shards internally, returns full output.

Strategy (node-per-partition): edges bucketed by destination node; each core
owns N/8 destination nodes, degree-sorted into blocks of 128 (one node per
SBUF partition). Per block, h_dst rows are broadcast-prefilled into SBUF and
an indirect DMA with accumulate adds gathered h_src rows, giving
s = h_src[j] + h_dst[i] per edge slot with no compute-engine pass.
att-weighted LeakyReLU reduces use LR(s) = 0.6 s + 0.4|s|: the linear term is
precomputed per node (extra row columns), the |s| term is two abs-reduces per
head over sign-partitioned channels prescaled by |0.4 att| (folded into the
projection weights). Aggregation: num = sum_e ex*s - den*h_dst. Softmax
max-subtraction is dropped (mathematically invariant; logits are O(1)).
"""
import os
import sys

for _p in ("/opt/trn_rl_repo", "/root/.axon_site/_ro/trn_rl_repo"):
    if os.path.isdir(_p) and _p not in sys.path:
        sys.path.insert(0, _p)

import numpy as np
import concourse.bass as bass
import concourse.bacc as bacc
import concourse.mybir as mybir
import concourse.tile as tile

P = 128
HEADS = 4
OUT_CH = 32
HC = HEADS * OUT_CH          # 128
EXT = HC + HEADS             # 132: h-channels + per-head base terms
EPS_BN = 1e-5

N_NODES = int(os.environ.get("GAT_N", 100000))
N_CORES = int(os.environ.get("GAT_CORES", 8))
R_CAP = int(os.environ.get("GAT_RCAP", 24))
RUN_MODE = os.environ.get("GAT_RUN", "hw")   # hw | sim
TRACE = os.environ.get("GAT_TRACE", "0") == "1"

NODES_PER_CORE = N_NODES // N_CORES
BLOCKS = (NODES_PER_CORE + P - 1) // P
NPAD = BLOCKS * P
XT_TILES = (N_NODES + P - 1) // P
XT_COLS = XT_TILES * P
SENT_ROW = XT_COLS           # sentinel row index in hsrc table

f32 = mybir.dt.float32
i32 = mybir.dt.int32

LAST_RESULT = {}             # exec_time_ns etc, for test harness introspection
_PROGRAM_CACHE = {}


def _host_prep(x, edge_index, W_src, W_dst, att):
    src = edge_index[0].astype(np.int64)
    dst = edge_index[1].astype(np.int64)
    loop = np.arange(N_NODES, dtype=np.int64)
    src2 = np.concatenate([src, loop])
    dst2 = np.concatenate([dst, loop])
    deg = np.bincount(dst2, minlength=N_NODES)
    order = np.argsort(dst2, kind="stable")
    src_sorted = src2[order].astype(np.int64)
    starts = np.zeros(N_NODES + 1, np.int64)
    starts[1:] = np.cumsum(deg)

    # per-core degree-sorted node permutation (pads replicate the core's
    # first node but get a single self-slot)
    perms = np.zeros((N_CORES, NPAD), np.int64)
    is_pad = np.zeros((N_CORES, NPAD), bool)
    for k in range(N_CORES):
        nodes = np.arange(k * NODES_PER_CORE, (k + 1) * NODES_PER_CORE)
        o = np.argsort(-deg[nodes], kind="stable")
        perms[k, :NODES_PER_CORE] = nodes[o]
        perms[k, NODES_PER_CORE:] = nodes[0]
        is_pad[k, NODES_PER_CORE:] = True

    degp = deg[perms]
    degp[is_pad] = 1
    degb = degp.reshape(N_CORES, BLOCKS, P)
    Rb = degb.max(axis=(0, 2)).astype(np.int64)   # uniform across cores

    rounds = []                                   # (block, r_off, rr)
    for b in range(BLOCKS):
        r, roff = int(Rb[b]), 0
        while r > 0:
            rr = min(r, R_CAP)
            rounds.append((b, roff, rr))
            roff += rr
            r -= rr
    tot = sum(rr for _, _, rr in rounds)

    idx_all = np.full((N_CORES, tot * P), SENT_ROW, np.int32)
    off = 0
    for (b, roff, rr) in rounds:
        for k in range(N_CORES):
            nodes = perms[k, b * P:(b + 1) * P]
            pad = is_pad[k, b * P:(b + 1) * P]
            nd = degp.reshape(N_CORES, NPAD)[k, b * P:(b + 1) * P]
            j = roff + np.arange(rr)[None, :]                   # [1, rr]
            base = np.where(pad, 0, starts[nodes])[:, None]
            gidx = np.clip(base + j, 0, src_sorted.size - 1)
            vals = src_sorted[gidx]
            vals = np.where(j < nd[:, None], vals, SENT_ROW)
            # pad nodes: single slot pointing at their own row
            vals = np.where((pad[:, None]) & (j == 0), nodes[:, None], vals)
            idx_all[k, off:off + P * rr] = vals.astype(np.int32).reshape(-1)
        off += P * rr

    # --- weights: channel perm (pos att first), |0.4 att| prescale ---
    att4 = 0.4 * att.astype(np.float64)
    cperm = np.zeros(HC, np.int64)
    scale = np.zeros(HC, np.float64)
    sbb = []
    for h in range(HEADS):
        pos = np.where(att4[h] > 0)[0]
        neg = np.where(att4[h] <= 0)[0]
        o = np.concatenate([pos, neg])
        sbb.append(len(pos))
        cperm[h * OUT_CH:(h + 1) * OUT_CH] = h * OUT_CH + o
        scale[h * OUT_CH:(h + 1) * OUT_CH] = np.abs(att4[h][o])
    scale = np.maximum(scale, 1e-30)

    def wext(W):
        Wt = W.astype(np.float64)[:, cperm] * scale[None, :]
        M = np.stack([W.astype(np.float64)[:, h * OUT_CH:(h + 1) * OUT_CH]
                      @ att[h].astype(np.float64) for h in range(HEADS)], axis=1)
        return np.concatenate([Wt, 0.6 * M], axis=1).astype(np.float32)

    wsrc_ext = wext(W_src)
    wdst_ext = wext(W_dst)
    chanscale = (1.0 / scale).astype(np.float32)

    xT = np.zeros((P, XT_COLS), np.float32)
    xT[:, :N_NODES] = x.T
    if XT_COLS > N_NODES:
        xT[:, N_NODES:] = x.T[:, :XT_COLS - N_NODES]
    xTp = np.stack([np.ascontiguousarray(x[perms[k]].T) for k in range(N_CORES)])

    sent = np.zeros((1, EXT), np.float32)
    sent[0, HC:] = -1e30
    cs_tile = np.tile(chanscale[None, :], (P, 1)).astype(np.float32)

    return dict(rounds=tuple(rounds), sbb=tuple(sbb), tot=tot,
                idx_all=idx_all, perms=perms, cperm=cperm,
                wsrc_ext=wsrc_ext, wdst_ext=wdst_ext, sent=sent,
                cs_tile=cs_tile, xT=xT, xTp=xTp)


def _build_program(rounds, sbb, tot):
    nc = bacc.Bacc("TRN2", target_bir_lowering=False, debug=False,
                   num_devices=N_CORES)
    xT = nc.dram_tensor("xT", [P, XT_COLS], f32, kind="ExternalInput")
    xTp = nc.dram_tensor("xTp", [P, NPAD], f32, kind="ExternalInput")
    wsrc = nc.dram_tensor("wsrc", [P, EXT], f32, kind="ExternalInput")
    wdst = nc.dram_tensor("wdst", [P, EXT], f32, kind="ExternalInput")
    sent = nc.dram_tensor("sent", [1, EXT], f32, kind="ExternalInput")
    cscale = nc.dram_tensor("cscale", [P, HC], f32, kind="ExternalInput")
    eidx = nc.dram_tensor("eidx", [tot * P], i32, kind="ExternalInput")
    y = nc.dram_tensor("y", [NPAD, HC], f32, kind="ExternalOutput")

    AX = mybir.AxisListType.X
    OP = mybir.AluOpType
    AF = mybir.ActivationFunctionType

    with tile.TileContext(nc) as tc:
        with (
            tc.tile_pool(name="dram", bufs=1, space="DRAM") as dp,
            tc.tile_pool(name="consts", bufs=1) as cp,
            tc.tile_pool(name="proj", bufs=4) as pp,
            tc.tile_pool(name="ppsum", bufs=4, space="PSUM") as pps,
            tc.tile_pool(name="edge", bufs=3) as ep,
            tc.tile_pool(name="small", bufs=3) as sp,
            tc.tile_pool(name="acc", bufs=2) as ap_,
        ):
            hsrc = dp.tile([XT_COLS + 1, EXT], f32)
            hdst = dp.tile([NPAD, EXT], f32)

            wsrc_t = cp.tile([P, EXT], f32)
            nc.sync.dma_start(out=wsrc_t[:], in_=wsrc[:])
            wdst_t = cp.tile([P, EXT], f32)
            nc.sync.dma_start(out=wdst_t[:], in_=wdst[:])
            cs_t = cp.tile([P, HC], f32)
            nc.sync.dma_start(out=cs_t[:], in_=cscale[:])
            sent_t = cp.tile([1, EXT], f32)
            nc.sync.dma_start(out=sent_t[:], in_=sent[:])
            nc.sync.dma_start(out=hsrc[SENT_ROW:SENT_ROW + 1, :], in_=sent_t[:])

            # ---- projections (batched: 4 node-tiles per DMA round-trip) ----
            def project(n_tiles, src_dram, w_tile, dst_dram):
                B = 4
                for t0 in range(0, n_tiles, B):
                    nb = min(B, n_tiles - t0)
                    xt = pp.tile([P, B * P], f32, tag="xt")
                    nc.sync.dma_start(
                        out=xt[:, :nb * P],
                        in_=src_dram[:, t0 * P:(t0 + nb) * P])
                    hs = pp.tile([P, B * EXT], f32, tag="hs")
                    for j in range(nb):
                        ps = pps.tile([P, EXT], f32, space="PSUM", tag="pps")
                        nc.tensor.matmul(out=ps[:],
                                         lhsT=xt[:, j * P:(j + 1) * P],
                                         rhs=w_tile[:], start=True, stop=True)
                        dst = hs[:, j * EXT:(j + 1) * EXT]
                        if j % 2 == 0:
                            nc.scalar.copy(out=dst, in_=ps[:])
                        else:
                            nc.vector.tensor_copy(out=dst, in_=ps[:])
                    # one store covering nb*128 rows
                    a = hs[:, :nb * EXT]
                    src_v = a.rearrange("p (j c) -> p j c", c=EXT)
                    d = dst_dram[t0 * P:(t0 + nb) * P, :]
                    dst_v = bass.AP(d.tensor, d.offset,
                                    [[EXT, P], [P * EXT, nb], [1, EXT]])
                    nc.sync.dma_start(out=dst_v, in_=src_v)

            project(XT_TILES, xT, wsrc_t, hsrc)
            project(BLOCKS, xTp, wdst_t, hdst)

            # ---- edge phase ----
            eoff = 0
            cur_b = -1
            hd_t = num_t = den_t = None
            n_in_block = {}
            for b, _, _ in rounds:
                n_in_block[b] = n_in_block.get(b, 0) + 1
            done_in_block = 0

            for (b, roff, rr) in rounds:
                first = b != cur_b
                if first:
                    cur_b = b
                    done_in_block = 0
                    hd_t = ep.tile([P, EXT], f32, tag="hd")
                    nc.sync.dma_start(out=hd_t[:], in_=hdst[b * P:(b + 1) * P, :])
                    num_t = ap_.tile([P, HC], f32, tag="num")
                    den_t = ap_.tile([P, HEADS], f32, tag="den")
                done_in_block += 1
                last = done_in_block == n_in_block[b]

                # prefill sum tile with h_dst broadcast, then gather-accumulate
                sum_t = ep.tile([P, R_CAP * EXT], f32, tag="sum")
                a = hd_t[:]
                hd_b = bass.AP(a.tensor, a.offset,
                               [list(a.ap[0]), [0, rr], list(a.ap[-1])])
                s3 = sum_t[:, :rr * EXT].rearrange("p (r c) -> p r c", c=EXT)
                nc.scalar.copy(out=s3, in_=hd_b)

                it = sp.tile([P, R_CAP], i32, tag="idx")
                nc.sync.dma_start(
                    out=it[:, :rr],
                    in_=eidx[eoff:eoff + P * rr].rearrange("(p r) -> p r", r=rr))
                eoff += P * rr
                # NOTE: multi-index-per-partition indirect DMA miscompiles on
                # HW (walrus lowers to first-index + sequential rows), so one
                # [P,1] gather-accumulate per slot.
                for r in range(rr):
                    nc.gpsimd.indirect_dma_start(
                        out=sum_t[:, r * EXT:(r + 1) * EXT], out_offset=None,
                        in_=hsrc[:],
                        in_offset=bass.IndirectOffsetOnAxis(
                            ap=it[:, r:r + 1], axis=0),
                        compute_op=OP.add)

                # per-(head, sign) abs-reduces -> lg [P, 8, rr]
                lg = sp.tile([P, 8 * R_CAP], f32, tag="lg")
                for h in range(HEADS):
                    for sgn in range(2):
                        c0 = h * OUT_CH + (0 if sgn == 0 else sbb[h])
                        c1 = h * OUT_CH + (sbb[h] if sgn == 0 else OUT_CH)
                        sl = lg[:, (h + 4 * sgn) * rr:(h + 4 * sgn + 1) * rr]
                        if c1 == c0:
                            nc.gpsimd.memset(sl, 0.0)
                        else:
                            nc.vector.reduce_sum(
                                out=sl.rearrange("p (r o) -> p r o", o=1),
                                in_=s3[:, :, c0:c1], axis=AX,
                                apply_absolute_value=True)

                # logits = base + pos - neg   [P, 4, rr] head-major
                base_v = sum_t[:, :rr * EXT].rearrange(
                    "p (r c) -> p c r", c=EXT)[:, HC:HC + HEADS, :]
                lg3 = lg[:, :8 * rr].rearrange("p (s r) -> p s r", r=rr)
                t1 = sp.tile([P, HEADS * R_CAP], f32, tag="t1")
                t1v = t1[:, :HEADS * rr].rearrange("p (h r) -> p h r", r=rr)
                nc.vector.tensor_tensor(out=t1v, in0=base_v, in1=lg3[:, 0:4, :],
                                        op=OP.add)
                lgt = sp.tile([P, HEADS * R_CAP], f32, tag="lgt")
                lgtv = lgt[:, :HEADS * rr].rearrange("p (h r) -> p h r", r=rr)
                nc.vector.tensor_tensor(out=lgtv, in0=t1v, in1=lg3[:, 4:8, :],
                                        op=OP.subtract)

                ex = sp.tile([P, HEADS * R_CAP], f32, tag="ex")
                nc.scalar.activation(out=ex[:, :HEADS * rr],
                                     in_=lgt[:, :HEADS * rr], func=AF.Exp)
                exv = ex[:, :HEADS * rr].rearrange("p (h r) -> p h r", r=rr)

                # den partial
                if first:
                    nc.vector.reduce_sum(
                        out=den_t[:].rearrange("p (h o) -> p h o", o=1),
                        in_=exv, axis=AX)
                else:
                    dtmp = sp.tile([P, HEADS], f32, tag="dtmp")
                    nc.vector.reduce_sum(
                        out=dtmp[:].rearrange("p (h o) -> p h o", o=1),
                        in_=exv, axis=AX)
                    nc.vector.tensor_tensor(out=den_t[:], in0=den_t[:],
                                            in1=dtmp[:], op=OP.add)

                # msg = ex * sum  (broadcast ex over the 32 channels per head)
                msg = ep.tile([P, R_CAP * HC], f32, tag="msg")
                m4 = msg[:, :rr * HC].rearrange("p (r h c) -> p r h c",
                                                h=HEADS, c=OUT_CH)
                s4 = sum_t[:, :rr * EXT].rearrange(
                    "p (r c) -> p r c", c=EXT)[:, :, :HC].rearrange(
                    "p r (h c) -> p r h c", c=OUT_CH)
                e = ex[:, :HEADS * rr]
                exb = bass.AP(e.tensor, e.offset,
                              [list(e.ap[0]), [1, rr], [rr, HEADS], [0, OUT_CH]])
                nc.vector.tensor_tensor(out=m4, in0=s4, in1=exb, op=OP.mult)

                # num partial: reduce msg over slots
                mv = msg[:, :rr * HC].rearrange("p (r c) -> p c r", c=HC)
                if first:
                    nc.vector.reduce_sum(
                        out=num_t[:].rearrange("p (c o) -> p c o", o=1),
                        in_=mv, axis=AX)
                else:
                    ntmp = sp.tile([P, HC], f32, tag="ntmp")
                    nc.vector.reduce_sum(
                        out=ntmp[:].rearrange("p (c o) -> p c o", o=1),
                        in_=mv, axis=AX)
                    nc.vector.tensor_tensor(out=num_t[:], in0=num_t[:],
                                            in1=ntmp[:], op=OP.add)

                if last:
                    # num -= den * h_dst ; y = num / den * chanscale
                    nden = sp.tile([P, HEADS], f32, tag="nden")
                    nc.vector.tensor_scalar_mul(nden[:], den_t[:], -1.0)
                    for h in range(HEADS):
                        hs = slice(h * OUT_CH, (h + 1) * OUT_CH)
                        nc.vector.scalar_tensor_tensor(
                            out=num_t[:, hs], in0=hd_t[:, hs],
                            scalar=nden[:, h:h + 1], in1=num_t[:, hs],
                            op0=OP.mult, op1=OP.add)
                    rden = sp.tile([P, HEADS], f32, tag="rden")
                    nc.vector.reciprocal(out=rden[:], in_=den_t[:])
                    yt = sp.tile([P, HC], f32, tag="yt")
                    for h in range(HEADS):
                        hs = slice(h * OUT_CH, (h + 1) * OUT_CH)
                        nc.vector.tensor_scalar(
                            out=yt[:, hs], in0=num_t[:, hs],
                            scalar1=rden[:, h:h + 1], scalar2=None,
                            op0=OP.mult)
                    nc.vector.tensor_tensor(out=yt[:], in0=yt[:], in1=cs_t[:],
                                            op=OP.mult)
                    nc.sync.dma_start(out=y[b * P:(b + 1) * P, :], in_=yt[:])

    nc.compile()
    return nc


def _run(nc, in_maps):
    if RUN_MODE == "sim":
        from concourse import bass_interp
        assert N_CORES == 1
        sim = bass_interp.CoreSim(nc)
        for name, arr in in_maps[0].items():
            sim.tensor(name)[:] = arr
        sim.simulate()
        return [{"y": np.array(sim.tensor("y"))}]
    from concourse.bass_utils import run_bass_kernel_spmd
    if TRACE:
        try:
            import axon_prof  # noqa: F401  (registers NTFF hook)
        except Exception:
            pass
    res = run_bass_kernel_spmd(nc, in_maps, list(range(N_CORES)), trace=TRACE)
    LAST_RESULT["exec_time_ns"] = res.exec_time_ns
    LAST_RESULT["res"] = res
    return res.results


def kernel(x, edge_index, W_src, W_dst, att, bias, bn_gamma, bn_beta):
    x = np.asarray(x, np.float32)
    edge_index = np.asarray(edge_index)
    prep = _host_prep(x, edge_index, np.asarray(W_src), np.asarray(W_dst),
                      np.asarray(att))

    key = (prep["rounds"], prep["sbb"])
    if key not in _PROGRAM_CACHE:
        _PROGRAM_CACHE[key] = _build_program(prep["rounds"], prep["sbb"],
                                             prep["tot"])
    nc = _PROGRAM_CACHE[key]

    in_maps = []
    for k in range(N_CORES):
        in_maps.append({
            "xT": prep["xT"],
            "xTp": prep["xTp"][k],
            "wsrc": prep["wsrc_ext"],
            "wdst": prep["wdst_ext"],
            "sent": prep["sent"],
            "cscale": prep["cs_tile"],
            "eidx": prep["idx_all"][k],
        })
    results = _run(nc, in_maps)

    out = np.zeros((N_NODES, HC), np.float32)
    for k in range(N_CORES):
        yk = np.asarray(results[k]["y"])[:NODES_PER_CORE]
        out[np.ix_(prep["perms"][k][:NODES_PER_CORE], prep["cperm"])] = yk

    # bias + BatchNorm (batch stats) + LeakyReLU(0.02) epilogue
    out = out + np.asarray(bias, np.float32)[None, :]
    mean = out.mean(axis=0)
    var = out.var(axis=0)
    yv = (np.asarray(bn_gamma, np.float32) * (out - mean)
          / np.sqrt(var + EPS_BN) + np.asarray(bn_beta, np.float32))
    return np.where(yv > 0, yv, 0.02 * yv).astype(np.float32)



# revision 2
# speedup vs baseline: 1.0099x; 1.0099x over previous
"""GATv2 layer on 8 Trainium2 NeuronCores (Bass/Tile).

Self-contained: takes full inputs, shards internally, returns full output.

Strategy (host-gathered edge streams + TensorE recompute): edges bucketed by
destination node; each core owns N/8 destination nodes, degree-sorted into
blocks of 128 (one node per SBUF partition). The host pre-gathers x[src] for
every edge slot into a sequential bf16 stream xg [128ch x tot*128] (slot-major
columns), so the device never does random-access DMA. Per 6-slot PSUM group
(two banks x 3 slots): six matmuls (lhsT = xg slot, rhs = W_src_ext) compute
h_src, then one accumulating matmul per bank (lhsT = x_dst block, rhs =
W_dst_ext replicated x3) adds h_dst, leaving s = h_src[j] + h_dst[i] in PSUM.
The scalar engine copies s to SBUF as bf16. att-weighted LeakyReLU reduces use
LR(s) = 0.6 s + 0.4|s|: the linear term comes from 4 extra projection columns
(base), the |s| term is two abs-reduces per head over sign-partitioned
channels prescaled by |0.4 att| (folded into the projection weights). Empty
slots are masked by a preloaded -1e30 logit-bias table (added on gpsimd).
Softmax ex is expanded per-channel on gpsimd so the msg multiply runs in the
DVE 2x bf16 mode; the num aggregation is identity-stationary matmuls
accumulating slot tiles in a per-block PSUM bank (num = sum_e ex*s - den*h_dst
finalized in batches of 8 blocks). Softmax max-subtraction is dropped
(mathematically invariant; logits are O(1)).
"""
import os
import sys

for _p in ("/opt/trn_rl_repo", "/root/.axon_site/_ro/trn_rl_repo"):
    if os.path.isdir(_p) and _p not in sys.path:
        sys.path.insert(0, _p)

def _ensure_ntff_hook():
    """Best-effort: make bass_utils' axon NTFF profiling importable when the
    image's antenv package lacks axon_hooks (timing degrades gracefully
    otherwise)."""
    try:
        import antenv.axon_hooks  # noqa: F401
        return
    except Exception:
        pass
    try:
        import types

        import antenv
        from trn_agent_boot.trn_boot import _ntff_profile_via_ctypes

        mod = types.ModuleType("antenv.axon_hooks")
        holder = {}
        mod.set_axon_ntff_profile_hook = lambda h: holder.update(h=h)
        mod.get_axon_ntff_profile_hook = lambda: holder.get("h")
        sys.modules["antenv.axon_hooks"] = mod
        antenv.axon_hooks = mod
        mod.set_axon_ntff_profile_hook(
            _ntff_profile_via_ctypes("/opt/axon/libaxon_pjrt.so"))
    except Exception:
        pass


_ensure_ntff_hook()

import numpy as np
import concourse.bass as bass
import concourse.bacc as bacc
import concourse.mybir as mybir
import concourse.tile as tile

try:
    import ml_dtypes
    BF16 = ml_dtypes.bfloat16
except Exception:  # pragma: no cover
    BF16 = None

P = 128
HEADS = 4
OUT_CH = 32
HC = HEADS * OUT_CH          # 128
EXT = HC + HEADS             # 132: h-channels + per-head base terms
EPS_BN = 1e-5
GROUP = 3                    # edge slots per PSUM bank group (3*132 <= 512)
FBATCH = 8                   # blocks per batched finalize

N_NODES = int(os.environ.get("GAT_N", 100000))
N_CORES = int(os.environ.get("GAT_CORES", 8))
R_CAP = int(os.environ.get("GAT_RCAP", 24))
RUN_MODE = os.environ.get("GAT_RUN", "hw")   # hw | sim
TRACE = os.environ.get("GAT_TRACE", "0") == "1"

NODES_PER_CORE = N_NODES // N_CORES
BLOCKS = (NODES_PER_CORE + P - 1) // P
NPAD = BLOCKS * P

f32 = mybir.dt.float32
bf16 = mybir.dt.bfloat16
i32 = mybir.dt.int32

NEG_BIG = -1e30

LAST_RESULT = {}             # exec_time_ns etc, for test harness introspection
_PROGRAM_CACHE = {}


def _host_prep(x, edge_index, W_src, W_dst, att):
    src = edge_index[0].astype(np.int64)
    dst = edge_index[1].astype(np.int64)
    loop = np.arange(N_NODES, dtype=np.int64)
    src2 = np.concatenate([src, loop])
    dst2 = np.concatenate([dst, loop])
    deg = np.bincount(dst2, minlength=N_NODES)
    order = np.argsort(dst2, kind="stable")
    src_sorted = src2[order].astype(np.int64)
    starts = np.zeros(N_NODES + 1, np.int64)
    starts[1:] = np.cumsum(deg)

    # per-core degree-sorted node permutation (pads replicate the core's
    # first node but get a single self-slot)
    perms = np.zeros((N_CORES, NPAD), np.int64)
    is_pad = np.zeros((N_CORES, NPAD), bool)
    for k in range(N_CORES):
        nodes = np.arange(k * NODES_PER_CORE, (k + 1) * NODES_PER_CORE)
        o = np.argsort(-deg[nodes], kind="stable")
        perms[k, :NODES_PER_CORE] = nodes[o]
        perms[k, NODES_PER_CORE:] = nodes[0]
        is_pad[k, NODES_PER_CORE:] = True

    degp = deg[perms]
    degp[is_pad] = 1
    degb = degp.reshape(N_CORES, BLOCKS, P)
    Rb = degb.max(axis=(0, 2)).astype(np.int64)   # uniform across cores

    rounds = []                                   # (block, r_off, rr)
    for b in range(BLOCKS):
        r, roff = int(Rb[b]), 0
        while r > 0:
            rr = min(r, R_CAP)
            rounds.append((b, roff, rr))
            roff += rr
            r -= rr
    tot = sum(rr for _, _, rr in rounds)

    # --- weights: channel perm (pos att first), |0.4 att| prescale ---
    att4 = 0.4 * att.astype(np.float64)
    cperm = np.zeros(HC, np.int64)
    scale = np.zeros(HC, np.float64)
    sbb = []
    for h in range(HEADS):
        pos = np.where(att4[h] > 0)[0]
        neg = np.where(att4[h] <= 0)[0]
        o = np.concatenate([pos, neg])
        sbb.append(len(pos))
        cperm[h * OUT_CH:(h + 1) * OUT_CH] = h * OUT_CH + o
        scale[h * OUT_CH:(h + 1) * OUT_CH] = np.abs(att4[h][o])
    scale = np.maximum(scale, 1e-30)

    def wext(W):
        Wt = W.astype(np.float64)[:, cperm] * scale[None, :]
        M = np.stack([W.astype(np.float64)[:, h * OUT_CH:(h + 1) * OUT_CH]
                      @ att[h].astype(np.float64) for h in range(HEADS)], axis=1)
        return np.concatenate([Wt, 0.6 * M], axis=1).astype(np.float32)

    wsrc_ext = wext(W_src).astype(BF16)
    wdst_ext = wext(W_dst).astype(BF16)
    wdst3 = np.tile(wdst_ext, (1, GROUP))
    chanscale = (1.0 / scale).astype(np.float32)
    cs_tile = np.tile(chanscale[None, :], (P, 1)).astype(np.float32)

    # --- per-core edge streams: pre-gathered x columns + logit bias ---
    x16T = np.zeros((P, N_NODES + 1), BF16)
    x16T[:, :N_NODES] = np.ascontiguousarray(x.T).astype(BF16)
    SENT = N_NODES

    xg_all = np.zeros((N_CORES, P, tot * P), BF16)
    bias_all = np.zeros((N_CORES, P, tot), np.float32)
    for k in range(N_CORES):
        soff = 0
        for (b, roff, rr) in rounds:
            nodes = perms[k, b * P:(b + 1) * P]
            pad = is_pad[k, b * P:(b + 1) * P]
            nd = degp[k, b * P:(b + 1) * P]
            j = roff + np.arange(rr)[None, :]                   # [1, rr]
            base = np.where(pad, 0, starts[nodes])[:, None]
            gidx = np.clip(base + j, 0, src_sorted.size - 1)
            vals = src_sorted[gidx]
            vals = np.where(j < nd[:, None], vals, SENT)
            # pad nodes: single slot pointing at their own row
            vals = np.where((pad[:, None]) & (j == 0), nodes[:, None], vals)
            xg_all[k, :, soff * P:(soff + rr) * P] = \
                x16T[:, vals.T.reshape(-1)]
            bias_all[k, :, soff:soff + rr] = \
                np.where(vals == SENT, NEG_BIG, 0.0).astype(np.float32)
            soff += rr

    xTp = np.stack([np.ascontiguousarray(x[perms[k]].T).astype(BF16)
                    for k in range(N_CORES)])
    ident = np.eye(P, dtype=np.float32).astype(BF16)

    return dict(rounds=tuple(rounds), sbb=tuple(sbb), tot=tot,
                perms=perms, cperm=cperm,
                wsrc_ext=wsrc_ext, wdst3=wdst3, ident=ident,
                cs_tile=cs_tile, xg_all=xg_all, bias_all=bias_all, xTp=xTp)


def _build_program(rounds, sbb, tot):
    nc = bacc.Bacc("TRN2", target_bir_lowering=False, debug=False,
                   num_devices=N_CORES)
    xTp = nc.dram_tensor("xTp", [P, NPAD], bf16, kind="ExternalInput")
    wsrc = nc.dram_tensor("wsrc", [P, EXT], bf16, kind="ExternalInput")
    wdst3d = nc.dram_tensor("wdst3", [P, GROUP * EXT], bf16,
                            kind="ExternalInput")
    identd = nc.dram_tensor("ident", [P, P], bf16, kind="ExternalInput")
    cscale = nc.dram_tensor("cscale", [P, HC], f32, kind="ExternalInput")
    xg = nc.dram_tensor("xg", [P, tot * P], bf16, kind="ExternalInput")
    biasd = nc.dram_tensor("biasd", [P, tot], f32, kind="ExternalInput")
    y = nc.dram_tensor("y", [NPAD, HC], f32, kind="ExternalOutput")

    AX = mybir.AxisListType.X
    OP = mybir.AluOpType
    AF = mybir.ActivationFunctionType

    n_in_block = {}
    for b, _, _ in rounds:
        n_in_block[b] = n_in_block.get(b, 0) + 1

    BANK = 512                   # f32 elements per PSUM bank
    with tile.TileContext(nc) as tc:
        with (
            tc.tile_pool(name="consts", bufs=1) as cp,
            tc.tile_pool(name="edgep", bufs=3, space="PSUM") as pps,
            tc.tile_pool(name="nump", bufs=2, space="PSUM") as npp,
            tc.tile_pool(name="xgp", bufs=4) as gp,
            tc.tile_pool(name="edge", bufs=4) as ep,
            tc.tile_pool(name="small", bufs=4) as sp,
            tc.tile_pool(name="fin", bufs=2) as fp,
            tc.tile_pool(name="acc", bufs=2) as ap_,
        ):
            wsrc_t = cp.tile([P, EXT], bf16)
            nc.sync.dma_start(out=wsrc_t[:], in_=wsrc[:])
            wdst3_t = cp.tile([P, GROUP * EXT], bf16)
            nc.sync.dma_start(out=wdst3_t[:], in_=wdst3d[:])
            id_t = cp.tile([P, P], bf16)
            nc.sync.dma_start(out=id_t[:], in_=identd[:])
            cs_t = cp.tile([P, HC], f32)
            nc.sync.dma_start(out=cs_t[:], in_=cscale[:])
            xtp_t = cp.tile([P, NPAD], bf16)
            nc.sync.dma_start(out=xtp_t[:], in_=xTp[:])
            bias_t = cp.tile([P, tot], f32)
            nc.sync.dma_start(out=bias_t[:], in_=biasd[:])
            hd_all = cp.tile([P, BLOCKS * EXT], f32)

            # ---- h_dst projection for all blocks (kept in SBUF) ----
            for b in range(BLOCKS):
                ps = pps.tile([P, 2 * BANK], f32, space="PSUM", tag="eps")
                nc.tensor.matmul(out=ps[:, :EXT],
                                 lhsT=xtp_t[:, b * P:(b + 1) * P],
                                 rhs=wdst3_t[:, :EXT], start=True, stop=True)
                nc.scalar.copy(out=hd_all[:, b * EXT:(b + 1) * EXT],
                               in_=ps[:, :EXT])

            # ---- edge phase ----
            soff = 0
            cur_b = -1
            num8 = den8 = None
            done_in_block = 0
            fin_blocks = []      # blocks accumulated in the current batch
            pending = None       # deferred num-accumulate work of prior round

            def emit_phase_b(pend):
                p_msg, p_nps, p_first, p_last, p_rr, p_i8, p_num8 = pend
                for r in range(p_rr):
                    nc.tensor.matmul(
                        out=p_nps[:, :HC], lhsT=id_t[:],
                        rhs=p_msg[:, r * HC:(r + 1) * HC],
                        start=(p_first and r == 0),
                        stop=(p_last and r == p_rr - 1),
                        skip_group_check=True)
                if p_last:
                    nc.scalar.copy(out=p_num8[:, p_i8 * HC:(p_i8 + 1) * HC],
                                   in_=p_nps[:, :HC])

            def flush(fin_blocks, num8, den8):
                nb = len(fin_blocks)
                g0 = fin_blocks[0]
                # views over the batch
                n3 = num8[:, :nb * HC].rearrange("p (j c) -> p j c", c=HC)
                a = hd_all[:, g0 * EXT:(g0 + nb) * EXT]
                hd3 = bass.AP(a.tensor, a.offset,
                              [list(a.ap[0]), [EXT, nb], [1, HC]])
                d = den8[:, :nb * HEADS]
                db = bass.AP(d.tensor, d.offset,
                             [list(d.ap[0]), [HEADS, nb], [1, HEADS],
                              [0, OUT_CH]])
                n4 = num8[:, :nb * HC].rearrange("p (j h c) -> p j h c",
                                                 h=HEADS, c=OUT_CH)
                tmp8 = fp.tile([P, FBATCH * HC], f32, tag="tmp8")
                t3 = tmp8[:, :nb * HC].rearrange("p (j h c) -> p j h c",
                                                 h=HEADS, c=OUT_CH)
                hd4 = bass.AP(a.tensor, a.offset,
                              [list(a.ap[0]), [EXT, nb], [OUT_CH, HEADS],
                               [1, OUT_CH]])
                nc.gpsimd.tensor_tensor(out=t3, in0=hd4, in1=db, op=OP.mult)
                nc.gpsimd.tensor_tensor(out=num8[:, :nb * HC],
                                        in0=num8[:, :nb * HC],
                                        in1=tmp8[:, :nb * HC],
                                        op=OP.subtract)
                rden = fp.tile([P, FBATCH * HEADS], f32, tag="rden")
                nc.vector.reciprocal(out=rden[:, :nb * HEADS],
                                     in_=den8[:, :nb * HEADS])
                r = rden[:, :nb * HEADS]
                rb = bass.AP(r.tensor, r.offset,
                             [list(r.ap[0]), [HEADS, nb], [1, HEADS],
                              [0, OUT_CH]])
                nc.gpsimd.tensor_tensor(out=n4, in0=n4, in1=rb, op=OP.mult)
                c = cs_t[:]
                cb = bass.AP(c.tensor, c.offset,
                             [list(c.ap[0]), [0, nb], [1, HC]])
                nc.gpsimd.tensor_tensor(out=n3, in0=n3, in1=cb, op=OP.mult)
                dy = y[g0 * P:(g0 + nb) * P, :]
                dst_v = bass.AP(dy.tensor, dy.offset,
                                [[HC, P], [P * HC, nb], [1, HC]])
                nc.sync.dma_start(out=dst_v, in_=n3)

            for (b, roff, rr) in rounds:
                first = b != cur_b
                if first:
                    if fin_blocks and fin_blocks[0] // FBATCH != b // FBATCH:
                        if pending is not None:
                            emit_phase_b(pending)
                            pending = None
                        flush(fin_blocks, num8, den8)
                        fin_blocks = []
                    if not fin_blocks:
                        num8 = ap_.tile([P, FBATCH * HC], f32, tag="num8")
                        den8 = ap_.tile([P, FBATCH * HEADS], f32, tag="den8")
                    fin_blocks.append(b)
                    cur_b = b
                    done_in_block = 0
                    num_ps = npp.tile([P, BANK], f32, space="PSUM",
                                      tag="nps")
                i8 = b % FBATCH
                done_in_block += 1
                last = done_in_block == n_in_block[b]

                xg_t = gp.tile([P, R_CAP * P], bf16, tag="xg")
                nc.sync.dma_start(out=xg_t[:, :rr * P],
                                  in_=xg[:, soff * P:(soff + rr) * P])

                # s = h_src + h_dst via PE; scalar engine copies to bf16.
                # PSUM tiles hold 2 banks = up to 6 slots (3 per bank).
                sum_t = ep.tile([P, R_CAP * EXT], bf16, tag="sum")
                ngroups = (rr + 2 * GROUP - 1) // (2 * GROUP)
                for g in range(ngroups):
                    ng = min(2 * GROUP, rr - g * 2 * GROUP)
                    ps = pps.tile([P, 2 * BANK], f32, space="PSUM",
                                  tag="eps")
                    for half in range(2):
                        nh = min(GROUP, ng - half * GROUP)
                        if nh <= 0:
                            break
                        for j in range(nh):
                            nc.tensor.matmul(
                                out=ps[:, half * BANK + j * EXT:
                                       half * BANK + (j + 1) * EXT],
                                lhsT=xg_t[:, (g * 2 * GROUP + half * GROUP
                                              + j) * P:
                                          (g * 2 * GROUP + half * GROUP
                                           + j + 1) * P],
                                rhs=wsrc_t[:], start=(j == 0), stop=False,
                                skip_group_check=True)
                        nc.tensor.matmul(
                            out=ps[:, half * BANK:half * BANK + nh * EXT],
                            lhsT=xtp_t[:, b * P:(b + 1) * P],
                            rhs=wdst3_t[:, :nh * EXT], start=False,
                            stop=True, skip_group_check=True)
                    # one strided copy over both banks
                    nh0 = min(GROUP, ng)
                    nh1 = ng - nh0
                    if nh1 > 0:
                        pa = ps[:]
                        pv = bass.AP(pa.tensor, pa.offset,
                                     [list(pa.ap[0]), [BANK, 2],
                                      [1, nh0 * EXT]])
                        if nh1 < nh0:
                            # ragged tail: copy the two banks separately
                            nc.scalar.copy(
                                out=sum_t[:, g * 2 * GROUP * EXT:
                                          (g * 2 * GROUP + nh0) * EXT],
                                in_=ps[:, :nh0 * EXT])
                            nc.scalar.copy(
                                out=sum_t[:, (g * 2 * GROUP + nh0) * EXT:
                                          (g * 2 * GROUP + ng) * EXT],
                                in_=ps[:, BANK:BANK + nh1 * EXT])
                        else:
                            sv = sum_t[:, g * 2 * GROUP * EXT:
                                       (g * 2 * GROUP + ng) * EXT]
                            s2 = sv.rearrange("p (k c) -> p k c",
                                              k=2, c=nh0 * EXT)
                            nc.scalar.copy(out=s2, in_=pv)
                    else:
                        nc.scalar.copy(
                            out=sum_t[:, g * 2 * GROUP * EXT:
                                      (g * 2 * GROUP + ng) * EXT],
                            in_=ps[:, :ng * EXT])

                if pending is not None:
                    emit_phase_b(pending)
                    pending = None

                s3 = sum_t[:, :rr * EXT].rearrange("p (r c) -> p r c", c=EXT)

                # per-(head, sign) abs-reduces -> lg [P, 8, rr]
                lg = sp.tile([P, 8 * R_CAP], f32, tag="lg")
                for h in range(HEADS):
                    for sgn in range(2):
                        c0 = h * OUT_CH + (0 if sgn == 0 else sbb[h])
                        c1 = h * OUT_CH + (sbb[h] if sgn == 0 else OUT_CH)
                        sl = lg[:, (h + 4 * sgn) * rr:(h + 4 * sgn + 1) * rr]
                        if c1 == c0:
                            nc.gpsimd.memset(sl, 0.0)
                        else:
                            nc.vector.reduce_sum(
                                out=sl.rearrange("p (r o) -> p r o", o=1),
                                in_=s3[:, :, c0:c1], axis=AX,
                                apply_absolute_value=True)

                # logits = base + pos - neg + mask_bias   [P, 4, rr]
                base_v = sum_t[:, :rr * EXT].rearrange(
                    "p (r c) -> p c r", c=EXT)[:, HC:HC + HEADS, :]
                lg3 = lg[:, :8 * rr].rearrange("p (s r) -> p s r", r=rr)
                t1 = sp.tile([P, HEADS * R_CAP], f32, tag="t1")
                t1v = t1[:, :HEADS * rr].rearrange("p (h r) -> p h r", r=rr)
                nc.gpsimd.tensor_tensor(out=t1v, in0=base_v,
                                        in1=lg3[:, 0:4, :], op=OP.add)
                lgt = sp.tile([P, HEADS * R_CAP], f32, tag="lgt")
                lgtv = lgt[:, :HEADS * rr].rearrange("p (h r) -> p h r", r=rr)
                nc.gpsimd.tensor_tensor(out=lgtv, in0=t1v, in1=lg3[:, 4:8, :],
                                        op=OP.subtract)
                bi = bias_t[:, soff:soff + rr]
                bib = bass.AP(bi.tensor, bi.offset,
                              [list(bi.ap[0]), [0, HEADS], [1, rr]])
                nc.gpsimd.tensor_tensor(out=lgtv, in0=lgtv, in1=bib,
                                        op=OP.add)

                # exp expanded 16-wide (broadcast read) so the msg
                # multiply runs in the DVE 2x mode in two chunks
                EW = OUT_CH // 2
                ex16 = ep.tile([P, R_CAP * HEADS * EW], bf16, tag="ex16")
                la = lgt[:, :HEADS * rr]
                lb = bass.AP(la.tensor, la.offset,
                             [list(la.ap[0]), [1, rr], [rr, HEADS],
                              [0, EW]])
                x4 = ex16[:, :rr * HEADS * EW].rearrange(
                    "p (r h c) -> p r h c", h=HEADS, c=EW)
                nc.scalar.activation(out=x4, in_=lb, func=AF.Exp)
                ea = ex16[:, :rr * HEADS * EW]
                exv = bass.AP(ea.tensor, ea.offset,
                              [list(ea.ap[0]), [EW, HEADS], [HEADS * EW, rr]])

                # den partial
                if first:
                    nc.vector.reduce_sum(
                        out=den8[:, i8 * HEADS:(i8 + 1) * HEADS].rearrange(
                            "p (h o) -> p h o", o=1),
                        in_=exv, axis=AX)
                else:
                    dtmp = sp.tile([P, HEADS], f32, tag="dtmp")
                    nc.vector.reduce_sum(
                        out=dtmp[:].rearrange("p (h o) -> p h o", o=1),
                        in_=exv, axis=AX)
                    nc.gpsimd.tensor_tensor(
                        out=den8[:, i8 * HEADS:(i8 + 1) * HEADS],
                        in0=den8[:, i8 * HEADS:(i8 + 1) * HEADS],
                        in1=dtmp[:], op=OP.add)

                # msg = ex * s  (all operands bf16, innermost contiguous
                # 16-wide; two chunks reuse the same ex16 tile)
                msg = ep.tile([P, R_CAP * HC], bf16, tag="msg")
                ma = msg[:]
                sa = sum_t[:]
                for kk in range(2):
                    mv = bass.AP(ma.tensor, ma.offset + kk * EW,
                                 [list(ma.ap[0]), [HC, rr],
                                  [OUT_CH, HEADS], [1, EW]])
                    sv = bass.AP(sa.tensor, sa.offset + kk * EW,
                                 [list(sa.ap[0]), [EXT, rr],
                                  [OUT_CH, HEADS], [1, EW]])
                    ev = bass.AP(ea.tensor, ea.offset,
                                 [list(ea.ap[0]), [HEADS * EW, rr],
                                  [EW, HEADS], [1, EW]])
                    nc.vector.tensor_tensor(out=mv, in0=sv, in1=ev,
                                            op=OP.mult)

                # num: accumulate slot tiles into the block PSUM bank via
                # identity-stationary matmuls. Deferred one round so the PE
                # stream never stalls on the DVE-produced msg tile.
                pending = (msg, num_ps, first, last, rr, i8, num8)

                soff += rr
            if pending is not None:
                emit_phase_b(pending)
            if fin_blocks:
                flush(fin_blocks, num8, den8)

    nc.compile()
    return nc


def _run(nc, in_maps):
    if RUN_MODE == "sim":
        from concourse import bass_interp
        assert N_CORES == 1
        sim = bass_interp.CoreSim(nc)
        for name, arr in in_maps[0].items():
            sim.tensor(name)[:] = arr
        sim.simulate()
        return [{"y": np.array(sim.tensor("y"))}]
    from concourse.bass_utils import run_bass_kernel_spmd
    res = run_bass_kernel_spmd(nc, in_maps, list(range(N_CORES)), trace=TRACE)
    LAST_RESULT["exec_time_ns"] = res.exec_time_ns
    LAST_RESULT["res"] = res
    return res.results


def kernel(x, edge_index, W_src, W_dst, att, bias, bn_gamma, bn_beta):
    x = np.asarray(x, np.float32)
    edge_index = np.asarray(edge_index)
    prep = _host_prep(x, edge_index, np.asarray(W_src), np.asarray(W_dst),
                      np.asarray(att))

    key = (prep["rounds"], prep["sbb"])
    if key not in _PROGRAM_CACHE:
        _PROGRAM_CACHE[key] = _build_program(prep["rounds"], prep["sbb"],
                                             prep["tot"])
    nc = _PROGRAM_CACHE[key]

    in_maps = []
    for k in range(N_CORES):
        in_maps.append({
            "xTp": prep["xTp"][k],
            "wsrc": prep["wsrc_ext"],
            "wdst3": prep["wdst3"],
            "ident": prep["ident"],
            "cscale": prep["cs_tile"],
            "xg": prep["xg_all"][k],
            "biasd": prep["bias_all"][k],
        })
    results = _run(nc, in_maps)

    out = np.zeros((N_NODES, HC), np.float32)
    for k in range(N_CORES):
        yk = np.asarray(results[k]["y"])[:NODES_PER_CORE]
        out[np.ix_(prep["perms"][k][:NODES_PER_CORE], prep["cperm"])] = yk

    # bias + BatchNorm (batch stats) + LeakyReLU(0.02) epilogue
    out = out + np.asarray(bias, np.float32)[None, :]
    mean = out.mean(axis=0)
    var = out.var(axis=0)
    yv = (np.asarray(bn_gamma, np.float32) * (out - mean)
          / np.sqrt(var + EPS_BN) + np.asarray(bn_beta, np.float32))
    return np.where(yv > 0, yv, 0.02 * yv).astype(np.float32)
